# revision 11
# baseline (speedup 1.0000x reference)
"""2-layer GAT (PyG GATConv, heads=1) on 8 Trainium2 NeuronCores.

Strategy (dst-owner sharding per spec sharding_hint), 3 NEFF launches with
host doing only data movement/layout between them:

  NEFF#1: per-core h1 = embed_chunk @ W1 (f32), s1/d1 = h1 @ a_{src,dst}1.
  host:   assembles full h1 table, expands PER-EDGE tensors by fancy-index
          (pure data movement): gt1[slot] = bf16(h1[src_e]) plus per-edge
          s1[src_e], d1[dst_e] and dst-column ids. Everything is packed
          partition-major so the device streams it SEQUENTIALLY (no
          dma_gather / SWDGE descriptors - that was the 8ns/descriptor
          bottleneck of the previous version).
  NEFF#2: layer-1 edge phase per core:
            w_e = exp(leakyrelu(s_e + d_e))          (3 whole-layer ops)
            msg = [h|1] * w  via ONE stride-0-broadcast tensor_tensor per
                  127-dst window (per-partition-scalar ops cost ~1-2us on
                  HW regardless of width - avoid them in hot loops!)
            psum[dst, {f,Z}] += O_g^T @ msg_g        (ONE matmul per group;
                  O = host-shipped raw one-hot; Z rides in the ones column)
            tail/window: rz=1/Z (per-dst = per-partition), relu via
                  max(x*rz, -b1) trick, transpose -> x2T; bias restored in
                  h2 = W2^T x2T + W2^T b1; s2/d2 = a2^T h2 -> host.
  NEFF#3: same on [h2|1], sigmoid tail -> [128, NW*64]; host unshuffles.

  Edges are grouped into 127-dst psum windows; group counts are maxed
  across cores so all 8 cores run one SPMD instruction stream.
"""
import sys

if '/opt/trn_rl_repo' not in sys.path:
    sys.path.insert(0, '/opt/trn_rl_repo')

import numpy as np
import ml_dtypes

from concourse import bacc, mybir
import concourse.tile as tile
from concourse.bass_utils import run_bass_kernel_spmd

BF16 = ml_dtypes.bfloat16
NCORES = 8
RESULTS = []  # BassKernelResults per NEFF launch (for test harness introspection)
WIN = 127          # dsts per psum window (col 127 = dummy slot for padding)
F32 = mybir.dt.float32
BF = mybir.dt.bfloat16
AF = mybir.ActivationFunctionType
OP = mybir.AluOpType


# ----------------------------------------------------------------- host pre
def _preprocess(edge_index, N):
    """Group edges by dst window, pad each (window) to a multiple of 128
    slots (counts maxed over cores for SPMD), and emit per-core slot->src,
    slot->dst, slot->dstcol arrays in partition-major [128, Gtot] layout."""
    CH = N // NCORES
    NW = -(-CH // WIN)
    src = np.concatenate([np.asarray(edge_index[0], np.int64),
                          np.arange(N, dtype=np.int64)])
    dst = np.concatenate([np.asarray(edge_index[1], np.int64),
                          np.arange(N, dtype=np.int64)])
    owner = dst // CH
    dl = dst - owner * CH

    percs = []
    cnt = np.zeros((NCORES, NW), np.int64)
    for c in range(NCORES):
        mc = owner == c
        cs, cd = src[mc], dl[mc]
        w = cd // WIN
        cnt[c] = np.bincount(w, minlength=NW)
        percs.append((cs, cd, w))
    G = -(-cnt.max(axis=0) // 128)          # groups per window, >=1
    base = np.zeros(NW + 1, np.int64)
    base[1:] = np.cumsum(128 * G)
    S = int(base[-1])
    Gtot = S // 128

    cores = []
    for c in range(NCORES):
        cs, cd, w = percs[c]
        order = np.argsort(w, kind='stable')
        cs, cd, w = cs[order], cd[order], w[order]
        cc = np.zeros(NW + 1, np.int64)
        cc[1:] = np.cumsum(cnt[c])
        rank = np.arange(len(cd)) - cc[w]
        slot = base[w] + rank
        srcslot = np.full(S, -1, np.int64)
        dstslot = np.full(S, -1, np.int64)
        colslot = np.full(S, -1, np.int64)    # -1 pad -> all-zero one-hot row
        srcslot[slot] = cs
        dstslot[slot] = cd + c * CH   # global dst id
        colslot[slot] = cd - w * WIN
        # partition-major: slot (g, p) -> [p, g]
        sidx = srcslot.reshape(Gtot, 128)            # [g, p] (slot-major)
        didx = dstslot.reshape(Gtot, 128)
        # one-hot rows O[p, g, c] = 1[colslot==c]; pad rows all-zero
        eye = np.zeros((129, 128), BF16)
        eye[np.arange(1, 128), np.arange(127)] = BF16(1.0)
        O = eye[colslot.reshape(Gtot, 128) + 1]      # [g, p, 128]
        O = np.ascontiguousarray(O.transpose(1, 0, 2)).reshape(128, Gtot * 128)
        cores.append(dict(sidx=sidx, didx=didx, O=O))
    return dict(CH=CH, NW=NW, G=G, Gtot=Gtot, cores=cores)


def _expand(tbl_pad, idx_gp):
    """tbl_pad: [N+1, F] (last row zeros). idx_gp: [Gtot, 128] with -1 pads.
    Returns partition-major [128, Gtot, F] contiguous."""
    idx = np.where(idx_gp < 0, tbl_pad.shape[0] - 1, idx_gp)
    out = tbl_pad[idx]                       # [Gtot, 128, F]
    return np.ascontiguousarray(out.transpose(1, 0, 2))


def _expand1(vec_pad, idx_gp):
    """vec_pad: [N+1] (last = 0). Returns [128, Gtot] f32 contiguous."""
    idx = np.where(idx_gp < 0, vec_pad.shape[0] - 1, idx_gp)
    return np.ascontiguousarray(vec_pad[idx].T.astype(np.float32))


# ------------------------------------------------------------------ NEFF #1
def _build_neff1(N, C, H, CH):
    nc = bacc.Bacc(None, target_bir_lowering=False)
    xT = nc.declare_dram_parameter("xT", [C, CH], BF, isOutput=False)
    W1 = nc.declare_dram_parameter("W1", [C, H], BF, isOutput=False)
    a1s = nc.declare_dram_parameter("a1s", [H, 1], BF, isOutput=False)
    a1d = nc.declare_dram_parameter("a1d", [H, 1], BF, isOutput=False)
    hb = nc.declare_dram_parameter("hb", [H, CH], BF, isOutput=True)
    s1o = nc.declare_dram_parameter("s1o", [1, CH], F32, isOutput=True)
    d1o = nc.declare_dram_parameter("d1o", [1, CH], F32, isOutput=True)

    KT = -(-C // 128)
    with tile.TileContext(nc) as tc:
        with tc.tile_pool(name="cst", bufs=1) as cp, \
             tc.tile_pool(name="wk", bufs=3) as wp, \
             tc.tile_pool(name="ps", bufs=2, space="PSUM") as pp, \
             tc.tile_pool(name="ps1", bufs=2, space="PSUM") as pp1:
            xts, w1s = [], []
            for k in range(KT):
                kc = min(128, C - 128 * k)
                xt = cp.tile([kc, CH], BF, tag=f"xt{k}")
                nc.sync.dma_start(out=xt[:], in_=xT[128 * k:128 * k + kc, :])
                w1 = cp.tile([kc, H], BF, tag=f"w1{k}")
                nc.sync.dma_start(out=w1[:], in_=W1[128 * k:128 * k + kc, :])
                xts.append(xt)
                w1s.append(w1)
            asb = cp.tile([H, 1], BF, tag="a1s")
            nc.sync.dma_start(out=asb[:], in_=a1s[:])
            adb = cp.tile([H, 1], BF, tag="a1d")
            nc.sync.dma_start(out=adb[:], in_=a1d[:])
            h1b = cp.tile([H, CH], BF, tag="h1b")

            CW = 500
            for o in range(0, CH, CW):
                cw = min(CW, CH - o)
                ph = pp.tile([H, CW], F32, space="PSUM", tag="ph")
                for k in range(KT):
                    nc.tensor.matmul(out=ph[:, :cw], lhsT=w1s[k][:],
                                     rhs=xts[k][:, o:o + cw],
                                     start=(k == 0), stop=(k == KT - 1))
                nc.scalar.activation(h1b[:, o:o + cw], ph[:, :cw], AF.Copy)
                nc.sync.dma_start(out=hb[:, o:o + cw], in_=h1b[:, o:o + cw])
            for o in range(0, CH, CW):
                cw = min(CW, CH - o)
                ps = pp1.tile([1, CW], F32, space="PSUM", tag="psv")
                nc.tensor.matmul(out=ps[:, :cw], lhsT=asb[:],
                                 rhs=h1b[:, o:o + cw], start=True, stop=True)
                sv = wp.tile([1, CW], F32, tag="sv")
                nc.vector.tensor_copy(out=sv[:, :cw], in_=ps[:, :cw])
                nc.sync.dma_start(out=s1o[:, o:o + cw], in_=sv[:, :cw])
                pd = pp1.tile([1, CW], F32, space="PSUM", tag="pdv")
                nc.tensor.matmul(out=pd[:, :cw], lhsT=adb[:],
                                 rhs=h1b[:, o:o + cw], start=True, stop=True)
                dv = wp.tile([1, CW], F32, tag="dv")
                nc.vector.tensor_copy(out=dv[:, :cw], in_=pd[:, :cw])
                nc.sync.dma_start(out=d1o[:, o:o + cw], in_=dv[:, :cw])
    nc.finalize()
    return nc


# --------------------------------------------------------- edge-phase NEFFs
def _build_edge_neff(layer, NW, G, Gtot, WT, H, K):
    """Per window: ONE broadcast tensor_tensor folds w into the [h|1] message
    rows (msg = gt * w), then one matmul per 128-edge group accumulates
    psum[dst, {f,Z}] += O_g^T @ msg_g  (O = raw one-hot, host-shipped).
    Tail: rz = 1/Z (per-partition = per-dst), then
      layer 1: xm = max(psum*rz, -b1) = relu(out+b1)-b1, transpose -> x2T;
               h2 = W2^T x2T + (W2^T b1), s2/d2 = a2^T h2.
      layer 2: sig = Sigmoid(psum*rz + b2) -> [128, NW*K] (host unshuffles).
    """
    FH = H if layer == 1 else K
    LC = FH + 1                          # gt cols per slot: [h | 1]
    maxG = int(G.max())

    nc = bacc.Bacc(None, target_bir_lowering=False)
    gt = nc.declare_dram_parameter("gt", [128, Gtot * LC], BF, isOutput=False)
    Od = nc.declare_dram_parameter("O", [128, Gtot * 128], BF, isOutput=False)
    se = nc.declare_dram_parameter("se", [128, Gtot], F32, isOutput=False)
    de = nc.declare_dram_parameter("de", [128, Gtot], F32, isOutput=False)
    brep = nc.declare_dram_parameter("brep", [128, FH], F32, isOutput=False)
    if layer == 1:
        W2 = nc.declare_dram_parameter("W2", [H, K], F32, isOutput=False)
        a2s = nc.declare_dram_parameter("a2s", [K, 1], F32, isOutput=False)
        a2d = nc.declare_dram_parameter("a2d", [K, 1], F32, isOutput=False)
        c2 = nc.declare_dram_parameter("c2", [K, 1], F32, isOutput=False)
        h2o = nc.declare_dram_parameter("h2o", [K, WT], BF, isOutput=True)
        s2o = nc.declare_dram_parameter("s2o", [1, WT], F32, isOutput=True)
        d2o = nc.declare_dram_parameter("d2o", [1, WT], F32, isOutput=True)
    else:
        outp = nc.declare_dram_parameter("out", [128, NW * K], F32,
                                         isOutput=True)

    from concourse.masks import make_identity
    with tile.TileContext(nc) as tc:
        with tc.tile_pool(name="cst", bufs=1) as cp:
            bsb = cp.tile([128, FH], F32, tag="br")
            nc.sync.dma_start(out=bsb[:], in_=brep[:])
            wv = cp.tile([128, Gtot], BF, tag="wv")
            with tc.tile_pool(name="sd", bufs=1) as sdp:
                sesb = sdp.tile([128, Gtot], F32, tag="se")
                nc.sync.dma_start(out=sesb[:], in_=se[:])
                desb = sdp.tile([128, Gtot], F32, tag="de")
                nc.sync.dma_start(out=desb[:], in_=de[:])
                zv = sdp.tile([128, Gtot], F32, tag="zv")
                nc.vector.tensor_tensor(out=zv[:], in0=sesb[:], in1=desb[:],
                                        op=OP.add)
                lr = sdp.tile([128, Gtot], F32, tag="lr")
                nc.vector.scalar_tensor_tensor(out=lr[:], in0=zv[:],
                                               scalar=0.2, in1=zv[:],
                                               op0=OP.mult, op1=OP.max)
                nc.scalar.activation(wv[:], lr[:], AF.Exp)
            if layer == 1:
                idn = cp.tile([128, 128], F32, tag="idn")
                make_identity(nc, idn[:])
                x2T = cp.tile([128, WT], F32, tag="x2T")
                w2sb = cp.tile([H, K], F32, tag="w2")
                nc.sync.dma_start(out=w2sb[:], in_=W2[:])
                a2ssb = cp.tile([K, 1], F32, tag="a2s")
                nc.sync.dma_start(out=a2ssb[:], in_=a2s[:])
                a2dsb = cp.tile([K, 1], F32, tag="a2d")
                nc.sync.dma_start(out=a2dsb[:], in_=a2d[:])
                c2sb = cp.tile([K, 1], F32, tag="c2")
                nc.sync.dma_start(out=c2sb[:], in_=c2[:])
            else:
                sgT = cp.tile([128, NW * K], F32, tag="sgT")

            with tc.tile_pool(name="gtp", bufs=3) as gtp, \
                 tc.tile_pool(name="otp", bufs=3) as otp, \
                 tc.tile_pool(name="msp", bufs=3) as msp, \
                 tc.tile_pool(name="wk", bufs=4) as wp, \
                 tc.tile_pool(name="psx", bufs=2, space="PSUM") as psxp, \
                 tc.tile_pool(name="ptr", bufs=2, space="PSUM") as ptrp:
                goff = 0
                for wdx in range(NW):
                    gw = int(G[wdx])
                    w0 = wdx * WIN
                    gtw = gtp.tile([128, maxG, LC], BF, tag="gt")
                    nc.sync.dma_start(
                        out=gtw[:, :gw, :],
                        in_=gt[:, goff * LC:(goff + gw) * LC])
                    ot = otp.tile([128, maxG, 128], BF, tag="ot")
                    nc.sync.dma_start(
                        out=ot[:, :gw, :],
                        in_=Od[:, goff * 128:(goff + gw) * 128])
                    msgw = msp.tile([128, maxG, LC], BF, tag="ms")
                    feng = nc.vector if (wdx % 2 == 0) else nc.gpsimd
                    feng.tensor_tensor(
                        out=msgw[:, :gw, :], in0=gtw[:, :gw, :],
                        in1=wv[:, goff:goff + gw].to_broadcast((128, gw, LC)),
                        op=OP.mult)
                    psx = psxp.tile([128, LC], F32, space="PSUM", tag="px")
                    for j in range(gw):
                        nc.tensor.matmul(out=psx[:], lhsT=ot[:, j, :],
                                         rhs=msgw[:, j, :],
                                         start=(j == 0), stop=(j == gw - 1))
                    # ---- window tail
                    rz = wp.tile([128, 1], F32, tag="rz")
                    nc.vector.reciprocal(out=rz[:], in_=psx[:, FH:FH + 1])
                    xm = wp.tile([128, FH], F32, tag="xm")
                    nc.vector.scalar_tensor_tensor(
                        out=xm[:], in0=psx[:, 0:FH], scalar=rz[:],
                        in1=bsb[:], op0=OP.mult,
                        op1=(OP.max if layer == 1 else OP.add))
                    if layer == 1:
                        pt = ptrp.tile([128, 128], F32, space="PSUM", tag="pt")
                        nc.tensor.transpose(pt[:], xm[:], idn[:])
                        nc.scalar.activation(x2T[:, w0:w0 + WIN],
                                             pt[:, 0:WIN], AF.Copy)
                    else:
                        nc.scalar.activation(sgT[:, wdx * K:(wdx + 1) * K],
                                             xm[:], AF.Sigmoid)
                    goff += gw

            if layer == 1:
                with tc.tile_pool(name="tl", bufs=3) as tp, \
                     tc.tile_pool(name="tc1", bufs=1) as tcp, \
                     tc.tile_pool(name="ph2", bufs=2, space="PSUM") as php, \
                     tc.tile_pool(name="psv", bufs=2, space="PSUM") as psp:
                    h2T = tcp.tile([K, WT], F32, tag="h2T")
                    CW = 512
                    for o in range(0, WT, CW):
                        cw = min(CW, WT - o)
                        ph = php.tile([K, CW], F32, space="PSUM", tag="ph")
                        nc.tensor.matmul(out=ph[:, :cw], lhsT=w2sb[:],
                                         rhs=x2T[:, o:o + cw],
                                         start=True, stop=True)
                        # h2 = W2^T xm^T + c2  (c2 = W2^T b1 restores bias)
                        nc.vector.tensor_scalar(
                            out=h2T[:, o:o + cw], in0=ph[:, :cw],
                            scalar1=c2sb[:], scalar2=None, op0=OP.add)
                        hh = tp.tile([K, CW], BF, tag="hh")
                        nc.scalar.activation(hh[:, :cw], h2T[:, o:o + cw],
                                             AF.Copy)
                        nc.sync.dma_start(out=h2o[:, o:o + cw], in_=hh[:, :cw])
                    for o in range(0, WT, CW):
                        cw = min(CW, WT - o)
                        ps = psp.tile([1, CW], F32, space="PSUM", tag="ps2")
                        nc.tensor.matmul(out=ps[:, :cw], lhsT=a2ssb[:],
                                         rhs=h2T[:, o:o + cw],
                                         start=True, stop=True)
                        sv = tp.tile([1, CW], F32, tag="sv")
                        nc.vector.tensor_copy(out=sv[:, :cw], in_=ps[:, :cw])
                        nc.sync.dma_start(out=s2o[:, o:o + cw], in_=sv[:, :cw])
                        pd = psp.tile([1, CW], F32, space="PSUM", tag="pd")
                        nc.tensor.matmul(out=pd[:, :cw], lhsT=a2dsb[:],
                                         rhs=h2T[:, o:o + cw],
                                         start=True, stop=True)
                        dv = tp.tile([1, CW], F32, tag="dv")
                        nc.vector.tensor_copy(out=dv[:, :cw], in_=pd[:, :cw])
                        nc.sync.dma_start(out=d2o[:, o:o + cw], in_=dv[:, :cw])
            else:
                nc.sync.dma_start(out=outp[:], in_=sgT[:])
    nc.finalize()
    return nc


# ------------------------------------------------------------------- driver
def kernel(edge_index, embed, W1, a_src1, a_dst1, b1, W2, a_src2, a_dst2, b2):
    RESULTS.clear()
    N, C = embed.shape
    H = W1.shape[1]
    K = W2.shape[1]
    CH = N // NCORES
    meta = _preprocess(np.asarray(edge_index), N)
    NW, G, Gtot = meta['NW'], meta['G'], meta['Gtot']
    WT = NW * WIN
    cores = list(range(NCORES))

    # ---- NEFF 1
    nc1 = _build_neff1(N, C, H, CH)
    maps1 = []
    for c in range(NCORES):
        xt = np.ascontiguousarray(embed[c * CH:(c + 1) * CH, :].T)
        maps1.append({"xT": xt.astype(BF16),
                      "W1": np.asarray(W1).astype(BF16),
                      "a1s": np.asarray(a_src1).astype(BF16)[:, None],
                      "a1d": np.asarray(a_dst1).astype(BF16)[:, None]})
    print("[kernel] NEFF1 built, running...", file=sys.stderr, flush=True)
    _res1 = run_bass_kernel_spmd(nc1, maps1, cores)
    RESULTS.append(_res1)
    r1 = _res1.results
    print("[kernel] NEFF1 done", file=sys.stderr, flush=True)

    # host: full [h1|1] (bf16), s1, d1 tables with zero pad row
    h1e = np.zeros((N + 1, H + 1), BF16)
    s1p = np.zeros(N + 1, np.float32)
    d1p = np.zeros(N + 1, np.float32)
    for c in range(NCORES):
        sl = slice(c * CH, (c + 1) * CH)
        h1e[sl, :H] = r1[c]["hb"].T
        h1e[sl, H] = BF16(1.0)
        s1p[sl.start:sl.stop] = r1[c]["s1o"][0]
        d1p[sl.start:sl.stop] = r1[c]["d1o"][0]

    # ---- NEFF 2
    nc2 = _build_edge_neff(1, NW, G, Gtot, WT, H, K)
    b1f = np.asarray(b1, np.float32)
    c2v = (np.asarray(W2, np.float32).T @ b1f)[:, None]
    maps2 = []
    for c in range(NCORES):
        m = meta['cores'][c]
        gt1 = _expand(h1e, m['sidx']).reshape(128, Gtot * (H + 1))
        maps2.append({
            "gt": gt1, "O": m['O'],
            "se": _expand1(s1p, m['sidx']),
            "de": _expand1(d1p, m['didx']),
            "brep": np.tile(-b1f, (128, 1)),
            "W2": np.asarray(W2, np.float32),
            "a2s": np.asarray(a_src2, np.float32)[:, None],
            "a2d": np.asarray(a_dst2, np.float32)[:, None],
            "c2": c2v})
    print("[kernel] NEFF2 built, running...", file=sys.stderr, flush=True)
    _res2 = run_bass_kernel_spmd(nc2, maps2, cores)
    RESULTS.append(_res2)
    r2 = _res2.results
    print("[kernel] NEFF2 done", file=sys.stderr, flush=True)

    # host: full [h2|1] (bf16), s2, d2 tables
    h2p = np.zeros((N + 1, K + 1), BF16)
    s2p = np.zeros(N + 1, np.float32)
    d2p = np.zeros(N + 1, np.float32)
    for c in range(NCORES):
        sl = slice(c * CH, (c + 1) * CH)
        h2p[sl, :K] = r2[c]["h2o"][:, :CH].T
        h2p[sl, K] = BF16(1.0)
        s2p[sl.start:sl.stop] = r2[c]["s2o"][0, :CH]
        d2p[sl.start:sl.stop] = r2[c]["d2o"][0, :CH]

    # ---- NEFF 3
    nc3 = _build_edge_neff(2, NW, G, Gtot, WT, H, K)
    maps3 = []
    for c in range(NCORES):
        m = meta['cores'][c]
        gt2 = _expand(h2p, m['sidx']).reshape(128, Gtot * (K + 1))
        maps3.append({
            "gt": gt2, "O": m['O'],
            "se": _expand1(s2p, m['sidx']),
            "de": _expand1(d2p, m['didx']),
            "brep": np.tile(np.asarray(b2, np.float32), (128, 1))})
    print("[kernel] NEFF3 built, running...", file=sys.stderr, flush=True)
    _res3 = run_bass_kernel_spmd(nc3, maps3, cores)
    RESULTS.append(_res3)
    r3 = _res3.results
    print("[kernel] NEFF3 done", file=sys.stderr, flush=True)

    # unshuffle: r3[c]["out"][p, w*K:(w+1)*K] is node c*CH + w*WIN + p (p<WIN)
    out = np.empty((N, K), np.float32)
    for c in range(NCORES):
        sg = r3[c]["out"].reshape(128, NW, K).transpose(1, 0, 2)  # [w, p, K]
        out[c * CH:(c + 1) * CH] = sg[:, :WIN, :].reshape(NW * WIN, K)[:CH]
    return out


# revision 12
# speedup vs baseline: 1.2968x; 1.2968x over previous
"""2-layer GAT (PyG GATConv, heads=1) on 8 Trainium2 NeuronCores.

Strategy (dst-owner sharding per spec sharding_hint), 3 NEFF launches with
host doing only data movement/layout between them:

  NEFF#1: per-core h1 = embed_chunk @ W1 (f32), s1/d1 = h1 @ a_{src,dst}1.
  host:   assembles full h1 table, expands PER-EDGE tensors by fancy-index
          (pure data movement): gt1[slot] = bf16(h1[src_e]) plus per-edge
          s1[src_e], d1[dst_e] and dst-column ids. Everything is packed
          partition-major so the device streams it SEQUENTIALLY (no
          dma_gather / SWDGE descriptors - that was the 8ns/descriptor
          bottleneck of the previous version).
  NEFF#2: layer-1 edge phase per core:
            w_e = exp(leakyrelu(s_e + d_e))          (3 whole-layer ops)
            msg = [h|1] * w  via ONE stride-0-broadcast tensor_tensor per
                  127-dst window (per-partition-scalar ops cost ~1-2us on
                  HW regardless of width - avoid them in hot loops!)
            psum[dst, {f,Z}] += O_g^T @ msg_g        (ONE matmul per group;
                  O = host-shipped raw one-hot; Z rides in the ones column)
            tail/window: rz=1/Z (per-dst = per-partition), relu via
                  max(x*rz, -b1) trick, transpose -> x2T; bias restored in
                  h2 = W2^T x2T + W2^T b1; s2/d2 = a2^T h2 -> host.
  NEFF#3: same on [h2|1], sigmoid tail -> [128, NW*64]; host unshuffles.

  Edges are grouped into 127-dst psum windows; group counts are maxed
  across cores so all 8 cores run one SPMD instruction stream.
"""
import sys

if '/opt/trn_rl_repo' not in sys.path:
    sys.path.insert(0, '/opt/trn_rl_repo')

import numpy as np
import ml_dtypes

from concourse import bacc, mybir
import concourse.tile as tile
from concourse.bass_utils import run_bass_kernel_spmd

BF16 = ml_dtypes.bfloat16
FP8 = ml_dtypes.float8_e4m3
NCORES = 8
RESULTS = []  # BassKernelResults per NEFF launch (for test harness introspection)
WIN = 127          # dsts per psum window (col 127 = dummy slot for padding)
F32 = mybir.dt.float32
BF = mybir.dt.bfloat16
F8 = mybir.dt.float8e4
AF = mybir.ActivationFunctionType
OP = mybir.AluOpType


# ----------------------------------------------------------------- host pre
def _preprocess(edge_index, N):
    """Group edges by dst window, pad each (window) to a multiple of 128
    slots (counts maxed over cores for SPMD), and emit per-core slot->src,
    slot->dst, slot->dstcol arrays in partition-major [128, Gtot] layout."""
    CH = N // NCORES
    NW = -(-CH // WIN)
    src = np.concatenate([np.asarray(edge_index[0], np.int64),
                          np.arange(N, dtype=np.int64)])
    dst = np.concatenate([np.asarray(edge_index[1], np.int64),
                          np.arange(N, dtype=np.int64)])
    owner = dst // CH
    dl = dst - owner * CH

    percs = []
    cnt = np.zeros((NCORES, NW), np.int64)
    for c in range(NCORES):
        mc = owner == c
        cs, cd = src[mc], dl[mc]
        w = cd // WIN
        cnt[c] = np.bincount(w, minlength=NW)
        percs.append((cs, cd, w))
    G = -(-cnt.max(axis=0) // 128)          # groups per window, >=1
    base = np.zeros(NW + 1, np.int64)
    base[1:] = np.cumsum(128 * G)
    S = int(base[-1])
    Gtot = S // 128

    cores = []
    for c in range(NCORES):
        cs, cd, w = percs[c]
        order = np.argsort(w, kind='stable')
        cs, cd, w = cs[order], cd[order], w[order]
        cc = np.zeros(NW + 1, np.int64)
        cc[1:] = np.cumsum(cnt[c])
        rank = np.arange(len(cd)) - cc[w]
        slot = base[w] + rank
        srcslot = np.full(S, -1, np.int64)
        dstslot = np.full(S, -1, np.int64)
        colslot = np.full(S, -1, np.int64)    # -1 pad -> all-zero one-hot row
        srcslot[slot] = cs
        dstslot[slot] = cd + c * CH   # global dst id
        colslot[slot] = cd - w * WIN
        # partition-major: slot (g, p) -> [p, g]
        sidx = srcslot.reshape(Gtot, 128)            # [g, p] (slot-major)
        didx = dstslot.reshape(Gtot, 128)
        # one-hot rows O[p, g, c] = 1[colslot==c]; pad rows all-zero
        eye = np.zeros((129, 128), FP8)
        eye[np.arange(1, 128), np.arange(127)] = FP8(1.0)
        O = eye[colslot.reshape(Gtot, 128) + 1]      # [g, p, 128]
        O = np.ascontiguousarray(O.transpose(1, 0, 2)).reshape(128, Gtot * 128)
        cores.append(dict(sidx=sidx, didx=didx, O=O))
    return dict(CH=CH, NW=NW, G=G, Gtot=Gtot, cores=cores)


def _expand(tbl_pad, idx_gp):
    """tbl_pad: [N+1, F] (last row zeros). idx_gp: [Gtot, 128] with -1 pads.
    Returns partition-major [128, Gtot, F] contiguous."""
    idx = np.where(idx_gp < 0, tbl_pad.shape[0] - 1, idx_gp)
    out = tbl_pad[idx]                       # [Gtot, 128, F]
    return np.ascontiguousarray(out.transpose(1, 0, 2))


def _expand1(vec_pad, idx_gp):
    """vec_pad: [N+1] (last = 0). Returns [128, Gtot] f32 contiguous."""
    idx = np.where(idx_gp < 0, vec_pad.shape[0] - 1, idx_gp)
    return np.ascontiguousarray(vec_pad[idx].T.astype(np.float32))


# ------------------------------------------------------------------ NEFF #1
def _build_neff1(N, C, H, CH):
    nc = bacc.Bacc(None, target_bir_lowering=False)
    xT = nc.declare_dram_parameter("xT", [C, CH], BF, isOutput=False)
    W1 = nc.declare_dram_parameter("W1", [C, H], BF, isOutput=False)
    a1s = nc.declare_dram_parameter("a1s", [H, 1], BF, isOutput=False)
    a1d = nc.declare_dram_parameter("a1d", [H, 1], BF, isOutput=False)
    hb = nc.declare_dram_parameter("hb", [H, CH], BF, isOutput=True)
    s1o = nc.declare_dram_parameter("s1o", [1, CH], F32, isOutput=True)
    d1o = nc.declare_dram_parameter("d1o", [1, CH], F32, isOutput=True)

    KT = -(-C // 128)
    with tile.TileContext(nc) as tc:
        with tc.tile_pool(name="cst", bufs=1) as cp, \
             tc.tile_pool(name="wk", bufs=3) as wp, \
             tc.tile_pool(name="ps", bufs=2, space="PSUM") as pp, \
             tc.tile_pool(name="ps1", bufs=2, space="PSUM") as pp1:
            xts, w1s = [], []
            for k in range(KT):
                kc = min(128, C - 128 * k)
                xt = cp.tile([kc, CH], BF, tag=f"xt{k}")
                nc.sync.dma_start(out=xt[:], in_=xT[128 * k:128 * k + kc, :])
                w1 = cp.tile([kc, H], BF, tag=f"w1{k}")
                nc.sync.dma_start(out=w1[:], in_=W1[128 * k:128 * k + kc, :])
                xts.append(xt)
                w1s.append(w1)
            asb = cp.tile([H, 1], BF, tag="a1s")
            nc.sync.dma_start(out=asb[:], in_=a1s[:])
            adb = cp.tile([H, 1], BF, tag="a1d")
            nc.sync.dma_start(out=adb[:], in_=a1d[:])
            h1b = cp.tile([H, CH], BF, tag="h1b")

            CW = 500
            for o in range(0, CH, CW):
                cw = min(CW, CH - o)
                ph = pp.tile([H, CW], F32, space="PSUM", tag="ph")
                for k in range(KT):
                    nc.tensor.matmul(out=ph[:, :cw], lhsT=w1s[k][:],
                                     rhs=xts[k][:, o:o + cw],
                                     start=(k == 0), stop=(k == KT - 1))
                nc.scalar.activation(h1b[:, o:o + cw], ph[:, :cw], AF.Copy)
                nc.sync.dma_start(out=hb[:, o:o + cw], in_=h1b[:, o:o + cw])
            for o in range(0, CH, CW):
                cw = min(CW, CH - o)
                ps = pp1.tile([1, CW], F32, space="PSUM", tag="psv")
                nc.tensor.matmul(out=ps[:, :cw], lhsT=asb[:],
                                 rhs=h1b[:, o:o + cw], start=True, stop=True)
                sv = wp.tile([1, CW], F32, tag="sv")
                nc.vector.tensor_copy(out=sv[:, :cw], in_=ps[:, :cw])
                nc.sync.dma_start(out=s1o[:, o:o + cw], in_=sv[:, :cw])
                pd = pp1.tile([1, CW], F32, space="PSUM", tag="pdv")
                nc.tensor.matmul(out=pd[:, :cw], lhsT=adb[:],
                                 rhs=h1b[:, o:o + cw], start=True, stop=True)
                dv = wp.tile([1, CW], F32, tag="dv")
                nc.vector.tensor_copy(out=dv[:, :cw], in_=pd[:, :cw])
                nc.sync.dma_start(out=d1o[:, o:o + cw], in_=dv[:, :cw])
    nc.finalize()
    return nc


# --------------------------------------------------------- edge-phase NEFFs
def _build_edge_neff(layer, NW, G, Gtot, WT, H, K):
    """Per window: ONE broadcast tensor_tensor folds w into the [h|1] message
    rows (msg = gt * w), then one matmul per 128-edge group accumulates
    psum[dst, {f,Z}] += O_g^T @ msg_g  (O = raw one-hot, host-shipped).
    Tail: rz = 1/Z (per-partition = per-dst), then
      layer 1: xm = max(psum*rz, -b1) = relu(out+b1)-b1, transpose -> x2T;
               h2 = W2^T x2T + (W2^T b1), s2/d2 = a2^T h2.
      layer 2: sig = Sigmoid(psum*rz + b2) -> [128, NW*K] (host unshuffles).
    """
    FH = H if layer == 1 else K
    LC = FH + 1                          # gt cols per slot: [h | 1]
    maxG = int(G.max())

    nc = bacc.Bacc(None, target_bir_lowering=False)
    gt = nc.declare_dram_parameter("gt", [128, Gtot * LC], BF, isOutput=False)
    Od = nc.declare_dram_parameter("O", [128, Gtot * 128], F8, isOutput=False)
    se = nc.declare_dram_parameter("se", [128, Gtot], F32, isOutput=False)
    de = nc.declare_dram_parameter("de", [128, Gtot], F32, isOutput=False)
    brep = nc.declare_dram_parameter("brep", [128, FH], F32, isOutput=False)
    if layer == 1:
        W2 = nc.declare_dram_parameter("W2", [H, K], F32, isOutput=False)
        a2s = nc.declare_dram_parameter("a2s", [K, 1], F32, isOutput=False)
        a2d = nc.declare_dram_parameter("a2d", [K, 1], F32, isOutput=False)
        c2 = nc.declare_dram_parameter("c2", [K, 1], F32, isOutput=False)
        h2o = nc.declare_dram_parameter("h2o", [K, WT], BF, isOutput=True)
        s2o = nc.declare_dram_parameter("s2o", [1, WT], F32, isOutput=True)
        d2o = nc.declare_dram_parameter("d2o", [1, WT], F32, isOutput=True)
    else:
        outp = nc.declare_dram_parameter("out", [128, NW * K], F32,
                                         isOutput=True)

    from concourse.masks import make_identity
    with tile.TileContext(nc) as tc:
        with tc.tile_pool(name="cst", bufs=1) as cp:
            bsb = cp.tile([128, FH], F32, tag="br")
            nc.sync.dma_start(out=bsb[:], in_=brep[:])
            wv = cp.tile([128, Gtot], BF, tag="wv")
            with tc.tile_pool(name="sd", bufs=1) as sdp:
                sesb = sdp.tile([128, Gtot], F32, tag="se")
                nc.sync.dma_start(out=sesb[:], in_=se[:])
                desb = sdp.tile([128, Gtot], F32, tag="de")
                nc.sync.dma_start(out=desb[:], in_=de[:])
                zv = sdp.tile([128, Gtot], F32, tag="zv")
                nc.vector.tensor_tensor(out=zv[:], in0=sesb[:], in1=desb[:],
                                        op=OP.add)
                lr = sdp.tile([128, Gtot], F32, tag="lr")
                nc.vector.scalar_tensor_tensor(out=lr[:], in0=zv[:],
                                               scalar=0.2, in1=zv[:],
                                               op0=OP.mult, op1=OP.max)
                nc.scalar.activation(wv[:], lr[:], AF.Exp)
            if layer == 1:
                idn = cp.tile([128, 128], F32, tag="idn")
                make_identity(nc, idn[:])
                x2T = cp.tile([128, WT], F32, tag="x2T")
                w2sb = cp.tile([H, K], F32, tag="w2")
                nc.sync.dma_start(out=w2sb[:], in_=W2[:])
                a2ssb = cp.tile([K, 1], F32, tag="a2s")
                nc.sync.dma_start(out=a2ssb[:], in_=a2s[:])
                a2dsb = cp.tile([K, 1], F32, tag="a2d")
                nc.sync.dma_start(out=a2dsb[:], in_=a2d[:])
                c2sb = cp.tile([K, 1], F32, tag="c2")
                nc.sync.dma_start(out=c2sb[:], in_=c2[:])
            else:
                sgT = cp.tile([128, NW * K], F32, tag="sgT")

            with tc.tile_pool(name="gtp", bufs=3) as gtp, \
                 tc.tile_pool(name="otp", bufs=3) as otp, \
                 tc.tile_pool(name="msp", bufs=3) as msp, \
                 tc.tile_pool(name="wk", bufs=4) as wp, \
                 tc.tile_pool(name="psx", bufs=2, space="PSUM") as psxp, \
                 tc.tile_pool(name="ptr", bufs=2, space="PSUM") as ptrp:
                goff = 0
                for wdx in range(NW):
                    gw = int(G[wdx])
                    w0 = wdx * WIN
                    gtw = gtp.tile([128, maxG, LC], BF, tag="gt")
                    nc.sync.dma_start(
                        out=gtw[:, :gw, :],
                        in_=gt[:, goff * LC:(goff + gw) * LC])
                    ot = otp.tile([128, maxG, 128], F8, tag="ot")
                    nc.sync.dma_start(
                        out=ot[:, :gw, :],
                        in_=Od[:, goff * 128:(goff + gw) * 128])
                    msgw = msp.tile([128, maxG, LC], BF, tag="ms")
                    feng = nc.vector if (wdx % 2 == 0) else nc.gpsimd
                    feng.tensor_tensor(
                        out=msgw[:, :gw, :], in0=gtw[:, :gw, :],
                        in1=wv[:, goff:goff + gw].to_broadcast((128, gw, LC)),
                        op=OP.mult)
                    psx = psxp.tile([128, LC], F32, space="PSUM", tag="px")
                    for j in range(gw):
                        nc.tensor.matmul(out=psx[:], lhsT=ot[:, j, :],
                                         rhs=msgw[:, j, :],
                                         start=(j == 0), stop=(j == gw - 1))
                    # ---- window tail
                    rz = wp.tile([128, 1], F32, tag="rz")
                    nc.vector.reciprocal(out=rz[:], in_=psx[:, FH:FH + 1])
                    xm = wp.tile([128, FH], F32, tag="xm")
                    nc.vector.scalar_tensor_tensor(
                        out=xm[:], in0=psx[:, 0:FH], scalar=rz[:],
                        in1=bsb[:], op0=OP.mult,
                        op1=(OP.max if layer == 1 else OP.add))
                    if layer == 1:
                        pt = ptrp.tile([128, 128], F32, space="PSUM", tag="pt")
                        nc.tensor.transpose(pt[:], xm[:], idn[:])
                        nc.scalar.activation(x2T[:, w0:w0 + WIN],
                                             pt[:, 0:WIN], AF.Copy)
                    else:
                        nc.scalar.activation(sgT[:, wdx * K:(wdx + 1) * K],
                                             xm[:], AF.Sigmoid)
                    goff += gw

            if layer == 1:
                with tc.tile_pool(name="tl", bufs=3) as tp, \
                     tc.tile_pool(name="tc1", bufs=1) as tcp, \
                     tc.tile_pool(name="ph2", bufs=2, space="PSUM") as php, \
                     tc.tile_pool(name="psv", bufs=2, space="PSUM") as psp:
                    h2T = tcp.tile([K, WT], F32, tag="h2T")
                    CW = 512
                    for o in range(0, WT, CW):
                        cw = min(CW, WT - o)
                        ph = php.tile([K, CW], F32, space="PSUM", tag="ph")
                        nc.tensor.matmul(out=ph[:, :cw], lhsT=w2sb[:],
                                         rhs=x2T[:, o:o + cw],
                                         start=True, stop=True)
                        # h2 = W2^T xm^T + c2  (c2 = W2^T b1 restores bias)
                        nc.vector.tensor_scalar(
                            out=h2T[:, o:o + cw], in0=ph[:, :cw],
                            scalar1=c2sb[:], scalar2=None, op0=OP.add)
                        hh = tp.tile([K, CW], BF, tag="hh")
                        nc.scalar.activation(hh[:, :cw], h2T[:, o:o + cw],
                                             AF.Copy)
                        nc.sync.dma_start(out=h2o[:, o:o + cw], in_=hh[:, :cw])
                    for o in range(0, WT, CW):
                        cw = min(CW, WT - o)
                        ps = psp.tile([1, CW], F32, space="PSUM", tag="ps2")
                        nc.tensor.matmul(out=ps[:, :cw], lhsT=a2ssb[:],
                                         rhs=h2T[:, o:o + cw],
                                         start=True, stop=True)
                        sv = tp.tile([1, CW], F32, tag="sv")
                        nc.vector.tensor_copy(out=sv[:, :cw], in_=ps[:, :cw])
                        nc.sync.dma_start(out=s2o[:, o:o + cw], in_=sv[:, :cw])
                        pd = psp.tile([1, CW], F32, space="PSUM", tag="pd")
                        nc.tensor.matmul(out=pd[:, :cw], lhsT=a2dsb[:],
                                         rhs=h2T[:, o:o + cw],
                                         start=True, stop=True)
                        dv = tp.tile([1, CW], F32, tag="dv")
                        nc.vector.tensor_copy(out=dv[:, :cw], in_=pd[:, :cw])
                        nc.sync.dma_start(out=d2o[:, o:o + cw], in_=dv[:, :cw])
            else:
                nc.sync.dma_start(out=outp[:], in_=sgT[:])
    nc.finalize()
    return nc


# ------------------------------------------------------------------- driver
def kernel(edge_index, embed, W1, a_src1, a_dst1, b1, W2, a_src2, a_dst2, b2):
    RESULTS.clear()
    N, C = embed.shape
    H = W1.shape[1]
    K = W2.shape[1]
    CH = N // NCORES
    meta = _preprocess(np.asarray(edge_index), N)
    NW, G, Gtot = meta['NW'], meta['G'], meta['Gtot']
    WT = NW * WIN
    cores = list(range(NCORES))

    # ---- NEFF 1
    nc1 = _build_neff1(N, C, H, CH)
    maps1 = []
    for c in range(NCORES):
        xt = np.ascontiguousarray(embed[c * CH:(c + 1) * CH, :].T)
        maps1.append({"xT": xt.astype(BF16),
                      "W1": np.asarray(W1).astype(BF16),
                      "a1s": np.asarray(a_src1).astype(BF16)[:, None],
                      "a1d": np.asarray(a_dst1).astype(BF16)[:, None]})
    print("[kernel] NEFF1 built, running...", file=sys.stderr, flush=True)
    _res1 = run_bass_kernel_spmd(nc1, maps1, cores)
    RESULTS.append(_res1)
    r1 = _res1.results
    print("[kernel] NEFF1 done", file=sys.stderr, flush=True)

    # host: full [h1|1] (bf16), s1, d1 tables with zero pad row
    h1e = np.zeros((N + 1, H + 1), BF16)
    s1p = np.zeros(N + 1, np.float32)
    d1p = np.zeros(N + 1, np.float32)
    for c in range(NCORES):
        sl = slice(c * CH, (c + 1) * CH)
        h1e[sl, :H] = r1[c]["hb"].T
        h1e[sl, H] = BF16(1.0)
        s1p[sl.start:sl.stop] = r1[c]["s1o"][0]
        d1p[sl.start:sl.stop] = r1[c]["d1o"][0]

    # ---- NEFF 2
    nc2 = _build_edge_neff(1, NW, G, Gtot, WT, H, K)
    b1f = np.asarray(b1, np.float32)
    c2v = (np.asarray(W2, np.float32).T @ b1f)[:, None]
    maps2 = []
    for c in range(NCORES):
        m = meta['cores'][c]
        gt1 = _expand(h1e, m['sidx']).reshape(128, Gtot * (H + 1))
        maps2.append({
            "gt": gt1, "O": m['O'],
            "se": _expand1(s1p, m['sidx']),
            "de": _expand1(d1p, m['didx']),
            "brep": np.tile(-b1f, (128, 1)),
            "W2": np.asarray(W2, np.float32),
            "a2s": np.asarray(a_src2, np.float32)[:, None],
            "a2d": np.asarray(a_dst2, np.float32)[:, None],
            "c2": c2v})
    print("[kernel] NEFF2 built, running...", file=sys.stderr, flush=True)
    _res2 = run_bass_kernel_spmd(nc2, maps2, cores)
    RESULTS.append(_res2)
    r2 = _res2.results
    print("[kernel] NEFF2 done", file=sys.stderr, flush=True)

    # host: full [h2|1] (bf16), s2, d2 tables
    h2p = np.zeros((N + 1, K + 1), BF16)
    s2p = np.zeros(N + 1, np.float32)
    d2p = np.zeros(N + 1, np.float32)
    for c in range(NCORES):
        sl = slice(c * CH, (c + 1) * CH)
        h2p[sl, :K] = r2[c]["h2o"][:, :CH].T
        h2p[sl, K] = BF16(1.0)
        s2p[sl.start:sl.stop] = r2[c]["s2o"][0, :CH]
        d2p[sl.start:sl.stop] = r2[c]["d2o"][0, :CH]

    # ---- NEFF 3
    nc3 = _build_edge_neff(2, NW, G, Gtot, WT, H, K)
    maps3 = []
    for c in range(NCORES):
        m = meta['cores'][c]
        gt2 = _expand(h2p, m['sidx']).reshape(128, Gtot * (K + 1))
        maps3.append({
            "gt": gt2, "O": m['O'],
            "se": _expand1(s2p, m['sidx']),
            "de": _expand1(d2p, m['didx']),
            "brep": np.tile(np.asarray(b2, np.float32), (128, 1))})
    print("[kernel] NEFF3 built, running...", file=sys.stderr, flush=True)
    _res3 = run_bass_kernel_spmd(nc3, maps3, cores)
    RESULTS.append(_res3)
    r3 = _res3.results
    print("[kernel] NEFF3 done", file=sys.stderr, flush=True)

    # unshuffle: r3[c]["out"][p, w*K:(w+1)*K] is node c*CH + w*WIN + p (p<WIN)
    out = np.empty((N, K), np.float32)
    for c in range(NCORES):
        sg = r3[c]["out"].reshape(128, NW, K).transpose(1, 0, 2)  # [w, p, K]
        out[c * CH:(c + 1) * CH] = sg[:, :WIN, :].reshape(NW * WIN, K)[:CH]
    return out


# revision 13
# speedup vs baseline: 1.3364x; 1.0305x over previous
"""2-layer GAT (PyG GATConv, heads=1) on 8 Trainium2 NeuronCores.

Strategy (dst-owner sharding per spec sharding_hint), 3 NEFF launches with
host doing only data movement/layout between them:

  NEFF#1: per-core h1 = embed_chunk @ W1 (f32), s1/d1 = h1 @ a_{src,dst}1.
  host:   assembles full h1 table, expands PER-EDGE tensors by fancy-index
          (pure data movement): gt1[slot] = bf16(h1[src_e]) plus per-edge
          s1[src_e], d1[dst_e] and dst-column ids. Everything is packed
          partition-major so the device streams it SEQUENTIALLY (no
          dma_gather / SWDGE descriptors - that was the 8ns/descriptor
          bottleneck of the previous version).
  NEFF#2: layer-1 edge phase per core:
            w_e = exp(leakyrelu(s_e + d_e))          (3 whole-layer ops)
            msg = [h|1] * w  via ONE stride-0-broadcast tensor_tensor per
                  127-dst window (per-partition-scalar ops cost ~1-2us on
                  HW regardless of width - avoid them in hot loops!)
            psum[dst, {f,Z}] += O_g^T @ msg_g        (ONE matmul per group;
                  O = host-shipped raw one-hot; Z rides in the ones column)
            tail/window: rz=1/Z (per-dst = per-partition), relu via
                  max(x*rz, -b1) trick, transpose -> x2T; bias restored in
                  h2 = W2^T x2T + W2^T b1; s2/d2 = a2^T h2 -> host.
  NEFF#3: same on [h2|1], sigmoid tail -> [128, NW*64]; host unshuffles.

  Edges are grouped into 127-dst psum windows; group counts are maxed
  across cores so all 8 cores run one SPMD instruction stream.
"""
import sys

if '/opt/trn_rl_repo' not in sys.path:
    sys.path.insert(0, '/opt/trn_rl_repo')

import numpy as np
import ml_dtypes

from concourse import bacc, mybir
import concourse.tile as tile
from concourse.bass_utils import run_bass_kernel_spmd

BF16 = ml_dtypes.bfloat16
FP8 = ml_dtypes.float8_e4m3
NCORES = 8
RESULTS = []  # BassKernelResults per NEFF launch (for test harness introspection)
WIN = 127          # dsts per psum window (col 127 = dummy slot for padding)
F32 = mybir.dt.float32
BF = mybir.dt.bfloat16
F8 = mybir.dt.float8e4
AF = mybir.ActivationFunctionType
OP = mybir.AluOpType


# ----------------------------------------------------------------- host pre
def _preprocess(edge_index, N):
    """Group edges by dst window, pad each (window) to a multiple of 128
    slots (counts maxed over cores for SPMD), and emit per-core slot->src,
    slot->dst, slot->dstcol arrays in partition-major [128, Gtot] layout."""
    CH = N // NCORES
    NW = -(-CH // WIN)
    src = np.concatenate([np.asarray(edge_index[0], np.int64),
                          np.arange(N, dtype=np.int64)])
    dst = np.concatenate([np.asarray(edge_index[1], np.int64),
                          np.arange(N, dtype=np.int64)])
    owner = dst // CH
    dl = dst - owner * CH

    percs = []
    cnt = np.zeros((NCORES, NW), np.int64)
    for c in range(NCORES):
        mc = owner == c
        cs, cd = src[mc], dl[mc]
        w = cd // WIN
        cnt[c] = np.bincount(w, minlength=NW)
        percs.append((cs, cd, w))
    G = -(-cnt.max(axis=0) // 128)          # groups per window, >=1
    base = np.zeros(NW + 1, np.int64)
    base[1:] = np.cumsum(128 * G)
    S = int(base[-1])
    Gtot = S // 128

    cores = []
    for c in range(NCORES):
        cs, cd, w = percs[c]
        order = np.argsort(w, kind='stable')
        cs, cd, w = cs[order], cd[order], w[order]
        cc = np.zeros(NW + 1, np.int64)
        cc[1:] = np.cumsum(cnt[c])
        rank = np.arange(len(cd)) - cc[w]
        slot = base[w] + rank
        srcslot = np.full(S, -1, np.int64)
        dstslot = np.full(S, -1, np.int64)
        colslot = np.full(S, -1, np.int64)    # -1 pad -> all-zero one-hot row
        srcslot[slot] = cs
        dstslot[slot] = cd + c * CH   # global dst id
        colslot[slot] = cd - w * WIN
        # partition-major: slot (g, p) -> [p, g]
        sidx = srcslot.reshape(Gtot, 128)            # [g, p] (slot-major)
        didx = dstslot.reshape(Gtot, 128)
        # one-hot rows O[p, g, c] = 1[colslot==c]; pad rows all-zero
        eye = np.zeros((129, 128), FP8)
        eye[np.arange(1, 128), np.arange(127)] = FP8(1.0)
        O = eye[colslot.reshape(Gtot, 128) + 1]      # [g, p, 128]
        O = np.ascontiguousarray(O.transpose(1, 0, 2)).reshape(128, Gtot * 128)
        cores.append(dict(sidx=sidx, didx=didx, O=O))
    return dict(CH=CH, NW=NW, G=G, Gtot=Gtot, cores=cores)


def _expand(tbl_pad, idx_gp):
    """tbl_pad: [N+1, F] (last row zeros). idx_gp: [Gtot, 128] with -1 pads.
    Returns partition-major [128, Gtot, F] contiguous."""
    idx = np.where(idx_gp < 0, tbl_pad.shape[0] - 1, idx_gp)
    out = tbl_pad[idx]                       # [Gtot, 128, F]
    return np.ascontiguousarray(out.transpose(1, 0, 2))


def _expand1(vec_pad, idx_gp):
    """vec_pad: [N+1] (last = 0). Returns [128, Gtot] f32 contiguous."""
    idx = np.where(idx_gp < 0, vec_pad.shape[0] - 1, idx_gp)
    return np.ascontiguousarray(vec_pad[idx].T.astype(np.float32))


# ------------------------------------------------------------------ NEFF #1
def _build_neff1(N, C, H, CH):
    nc = bacc.Bacc(None, target_bir_lowering=False)
    xT = nc.declare_dram_parameter("xT", [C, CH], BF, isOutput=False)
    W1 = nc.declare_dram_parameter("W1", [C, H], BF, isOutput=False)
    a1s = nc.declare_dram_parameter("a1s", [H, 1], BF, isOutput=False)
    a1d = nc.declare_dram_parameter("a1d", [H, 1], BF, isOutput=False)
    hb = nc.declare_dram_parameter("hb", [H, CH], BF, isOutput=True)
    s1o = nc.declare_dram_parameter("s1o", [1, CH], F32, isOutput=True)
    d1o = nc.declare_dram_parameter("d1o", [1, CH], F32, isOutput=True)

    KT = -(-C // 128)
    with tile.TileContext(nc) as tc:
        with tc.tile_pool(name="cst", bufs=1) as cp, \
             tc.tile_pool(name="wk", bufs=3) as wp, \
             tc.tile_pool(name="ps", bufs=2, space="PSUM") as pp, \
             tc.tile_pool(name="ps1", bufs=2, space="PSUM") as pp1:
            xts, w1s = [], []
            for k in range(KT):
                kc = min(128, C - 128 * k)
                xt = cp.tile([kc, CH], BF, tag=f"xt{k}")
                nc.sync.dma_start(out=xt[:], in_=xT[128 * k:128 * k + kc, :])
                w1 = cp.tile([kc, H], BF, tag=f"w1{k}")
                nc.sync.dma_start(out=w1[:], in_=W1[128 * k:128 * k + kc, :])
                xts.append(xt)
                w1s.append(w1)
            asb = cp.tile([H, 1], BF, tag="a1s")
            nc.sync.dma_start(out=asb[:], in_=a1s[:])
            adb = cp.tile([H, 1], BF, tag="a1d")
            nc.sync.dma_start(out=adb[:], in_=a1d[:])
            h1b = cp.tile([H, CH], BF, tag="h1b")

            CW = 500
            for o in range(0, CH, CW):
                cw = min(CW, CH - o)
                ph = pp.tile([H, CW], F32, space="PSUM", tag="ph")
                for k in range(KT):
                    nc.tensor.matmul(out=ph[:, :cw], lhsT=w1s[k][:],
                                     rhs=xts[k][:, o:o + cw],
                                     start=(k == 0), stop=(k == KT - 1))
                nc.scalar.activation(h1b[:, o:o + cw], ph[:, :cw], AF.Copy)
                nc.sync.dma_start(out=hb[:, o:o + cw], in_=h1b[:, o:o + cw])
            for o in range(0, CH, CW):
                cw = min(CW, CH - o)
                ps = pp1.tile([1, CW], F32, space="PSUM", tag="psv")
                nc.tensor.matmul(out=ps[:, :cw], lhsT=asb[:],
                                 rhs=h1b[:, o:o + cw], start=True, stop=True)
                sv = wp.tile([1, CW], F32, tag="sv")
                nc.vector.tensor_copy(out=sv[:, :cw], in_=ps[:, :cw])
                nc.sync.dma_start(out=s1o[:, o:o + cw], in_=sv[:, :cw])
                pd = pp1.tile([1, CW], F32, space="PSUM", tag="pdv")
                nc.tensor.matmul(out=pd[:, :cw], lhsT=adb[:],
                                 rhs=h1b[:, o:o + cw], start=True, stop=True)
                dv = wp.tile([1, CW], F32, tag="dv")
                nc.vector.tensor_copy(out=dv[:, :cw], in_=pd[:, :cw])
                nc.sync.dma_start(out=d1o[:, o:o + cw], in_=dv[:, :cw])
    nc.finalize()
    return nc


# --------------------------------------------------------- edge-phase NEFFs
def _build_edge_neff(layer, NW, G, Gtot, WT, H, K):
    """Per window: ONE broadcast tensor_tensor folds w into the [h|1] message
    rows (msg = gt * w), then one matmul per 128-edge group accumulates
    psum[dst, {f,Z}] += O_g^T @ msg_g  (O = raw one-hot, host-shipped).
    Tail: rz = 1/Z (per-partition = per-dst), then
      layer 1: xm = max(psum*rz, -b1) = relu(out+b1)-b1, transpose -> x2T;
               h2 = W2^T x2T + (W2^T b1), s2/d2 = a2^T h2.
      layer 2: sig = Sigmoid(psum*rz + b2) -> [128, NW*K] (host unshuffles).
    """
    FH = H if layer == 1 else K
    LC = FH + 1                          # gt cols per slot: [h | 1]
    maxG = int(G.max())

    nc = bacc.Bacc(None, target_bir_lowering=False)
    GDT = BF if layer == 1 else F8
    gt = nc.declare_dram_parameter("gt", [128, Gtot * LC], GDT, isOutput=False)
    Od = nc.declare_dram_parameter("O", [128, Gtot * 128], F8, isOutput=False)
    se = nc.declare_dram_parameter("se", [128, Gtot], F32, isOutput=False)
    de = nc.declare_dram_parameter("de", [128, Gtot], F32, isOutput=False)
    brep = nc.declare_dram_parameter("brep", [128, FH], F32, isOutput=False)
    if layer == 1:
        W2 = nc.declare_dram_parameter("W2", [H, K], F32, isOutput=False)
        a2s = nc.declare_dram_parameter("a2s", [K, 1], F32, isOutput=False)
        a2d = nc.declare_dram_parameter("a2d", [K, 1], F32, isOutput=False)
        c2 = nc.declare_dram_parameter("c2", [K, 1], F32, isOutput=False)
        h2o = nc.declare_dram_parameter("h2o", [K, WT], BF, isOutput=True)
        s2o = nc.declare_dram_parameter("s2o", [1, WT], F32, isOutput=True)
        d2o = nc.declare_dram_parameter("d2o", [1, WT], F32, isOutput=True)
    else:
        outp = nc.declare_dram_parameter("out", [128, NW * K], F32,
                                         isOutput=True)

    from concourse.masks import make_identity
    with tile.TileContext(nc) as tc:
        with tc.tile_pool(name="cst", bufs=1) as cp:
            bsb = cp.tile([128, FH], F32, tag="br")
            nc.sync.dma_start(out=bsb[:], in_=brep[:])
            wv = cp.tile([128, Gtot], BF, tag="wv")
            with tc.tile_pool(name="sd", bufs=1) as sdp:
                sesb = sdp.tile([128, Gtot], F32, tag="se")
                nc.sync.dma_start(out=sesb[:], in_=se[:])
                desb = sdp.tile([128, Gtot], F32, tag="de")
                nc.sync.dma_start(out=desb[:], in_=de[:])
                zv = sdp.tile([128, Gtot], F32, tag="zv")
                nc.vector.tensor_tensor(out=zv[:], in0=sesb[:], in1=desb[:],
                                        op=OP.add)
                lr = sdp.tile([128, Gtot], F32, tag="lr")
                nc.vector.scalar_tensor_tensor(out=lr[:], in0=zv[:],
                                               scalar=0.2, in1=zv[:],
                                               op0=OP.mult, op1=OP.max)
                nc.scalar.activation(wv[:], lr[:], AF.Exp)
            if layer == 1:
                idn = cp.tile([128, 128], F32, tag="idn")
                make_identity(nc, idn[:])
                x2T = cp.tile([128, WT], F32, tag="x2T")
                w2sb = cp.tile([H, K], F32, tag="w2")
                nc.sync.dma_start(out=w2sb[:], in_=W2[:])
                a2ssb = cp.tile([K, 1], F32, tag="a2s")
                nc.sync.dma_start(out=a2ssb[:], in_=a2s[:])
                a2dsb = cp.tile([K, 1], F32, tag="a2d")
                nc.sync.dma_start(out=a2dsb[:], in_=a2d[:])
                c2sb = cp.tile([K, 1], F32, tag="c2")
                nc.sync.dma_start(out=c2sb[:], in_=c2[:])
            else:
                sgT = cp.tile([128, NW * K], F32, tag="sgT")

            with tc.tile_pool(name="gtp", bufs=3) as gtp, \
                 tc.tile_pool(name="otp", bufs=3) as otp, \
                 tc.tile_pool(name="msp", bufs=3) as msp, \
                 tc.tile_pool(name="wk", bufs=4) as wp, \
                 tc.tile_pool(name="psx", bufs=2, space="PSUM") as psxp, \
                 tc.tile_pool(name="ptr", bufs=2, space="PSUM") as ptrp:
                goff = 0
                for wdx in range(NW):
                    gw = int(G[wdx])
                    w0 = wdx * WIN
                    gtw = gtp.tile([128, maxG, LC], GDT, tag="gt")
                    nc.sync.dma_start(
                        out=gtw[:, :gw, :],
                        in_=gt[:, goff * LC:(goff + gw) * LC])
                    ot = otp.tile([128, maxG, 128], F8, tag="ot")
                    nc.sync.dma_start(
                        out=ot[:, :gw, :],
                        in_=Od[:, goff * 128:(goff + gw) * 128])
                    msgw = msp.tile([128, maxG, LC], BF, tag="ms")
                    feng = nc.vector if (wdx % 2 == 0) else nc.gpsimd
                    feng.tensor_tensor(
                        out=msgw[:, :gw, :], in0=gtw[:, :gw, :],
                        in1=wv[:, goff:goff + gw].to_broadcast((128, gw, LC)),
                        op=OP.mult)
                    psx = psxp.tile([128, LC], F32, space="PSUM", tag="px")
                    for j in range(gw):
                        nc.tensor.matmul(out=psx[:], lhsT=ot[:, j, :],
                                         rhs=msgw[:, j, :],
                                         start=(j == 0), stop=(j == gw - 1))
                    # ---- window tail
                    rz = wp.tile([128, 1], F32, tag="rz")
                    nc.vector.reciprocal(out=rz[:], in_=psx[:, FH:FH + 1])
                    xm = wp.tile([128, FH], F32, tag="xm")
                    nc.vector.scalar_tensor_tensor(
                        out=xm[:], in0=psx[:, 0:FH], scalar=rz[:],
                        in1=bsb[:], op0=OP.mult,
                        op1=(OP.max if layer == 1 else OP.add))
                    if layer == 1:
                        pt = ptrp.tile([128, 128], F32, space="PSUM", tag="pt")
                        nc.tensor.transpose(pt[:], xm[:], idn[:])
                        nc.scalar.activation(x2T[:, w0:w0 + WIN],
                                             pt[:, 0:WIN], AF.Copy)
                    else:
                        nc.scalar.activation(sgT[:, wdx * K:(wdx + 1) * K],
                                             xm[:], AF.Sigmoid)
                    goff += gw

            if layer == 1:
                with tc.tile_pool(name="tl", bufs=3) as tp, \
                     tc.tile_pool(name="tc1", bufs=1) as tcp, \
                     tc.tile_pool(name="ph2", bufs=2, space="PSUM") as php, \
                     tc.tile_pool(name="psv", bufs=2, space="PSUM") as psp:
                    h2T = tcp.tile([K, WT], F32, tag="h2T")
                    CW = 512
                    for o in range(0, WT, CW):
                        cw = min(CW, WT - o)
                        ph = php.tile([K, CW], F32, space="PSUM", tag="ph")
                        nc.tensor.matmul(out=ph[:, :cw], lhsT=w2sb[:],
                                         rhs=x2T[:, o:o + cw],
                                         start=True, stop=True)
                        # h2 = W2^T xm^T + c2  (c2 = W2^T b1 restores bias)
                        nc.vector.tensor_scalar(
                            out=h2T[:, o:o + cw], in0=ph[:, :cw],
                            scalar1=c2sb[:], scalar2=None, op0=OP.add)
                        hh = tp.tile([K, CW], BF, tag="hh")
                        nc.scalar.activation(hh[:, :cw], h2T[:, o:o + cw],
                                             AF.Copy)
                        nc.sync.dma_start(out=h2o[:, o:o + cw], in_=hh[:, :cw])
                    for o in range(0, WT, CW):
                        cw = min(CW, WT - o)
                        ps = psp.tile([1, CW], F32, space="PSUM", tag="ps2")
                        nc.tensor.matmul(out=ps[:, :cw], lhsT=a2ssb[:],
                                         rhs=h2T[:, o:o + cw],
                                         start=True, stop=True)
                        sv = tp.tile([1, CW], F32, tag="sv")
                        nc.vector.tensor_copy(out=sv[:, :cw], in_=ps[:, :cw])
                        nc.sync.dma_start(out=s2o[:, o:o + cw], in_=sv[:, :cw])
                        pd = psp.tile([1, CW], F32, space="PSUM", tag="pd")
                        nc.tensor.matmul(out=pd[:, :cw], lhsT=a2dsb[:],
                                         rhs=h2T[:, o:o + cw],
                                         start=True, stop=True)
                        dv = tp.tile([1, CW], F32, tag="dv")
                        nc.vector.tensor_copy(out=dv[:, :cw], in_=pd[:, :cw])
                        nc.sync.dma_start(out=d2o[:, o:o + cw], in_=dv[:, :cw])
            else:
                nc.sync.dma_start(out=outp[:], in_=sgT[:])
    nc.finalize()
    return nc


# ------------------------------------------------------------------- driver
def kernel(edge_index, embed, W1, a_src1, a_dst1, b1, W2, a_src2, a_dst2, b2):
    RESULTS.clear()
    N, C = embed.shape
    H = W1.shape[1]
    K = W2.shape[1]
    CH = N // NCORES
    meta = _preprocess(np.asarray(edge_index), N)
    NW, G, Gtot = meta['NW'], meta['G'], meta['Gtot']
    WT = NW * WIN
    cores = list(range(NCORES))

    # ---- NEFF 1
    nc1 = _build_neff1(N, C, H, CH)
    maps1 = []
    for c in range(NCORES):
        xt = np.ascontiguousarray(embed[c * CH:(c + 1) * CH, :].T)
        maps1.append({"xT": xt.astype(BF16),
                      "W1": np.asarray(W1).astype(BF16),
                      "a1s": np.asarray(a_src1).astype(BF16)[:, None],
                      "a1d": np.asarray(a_dst1).astype(BF16)[:, None]})
    print("[kernel] NEFF1 built, running...", file=sys.stderr, flush=True)
    _res1 = run_bass_kernel_spmd(nc1, maps1, cores)
    RESULTS.append(_res1)
    r1 = _res1.results
    print("[kernel] NEFF1 done", file=sys.stderr, flush=True)

    # host: full [h1|1] (bf16), s1, d1 tables with zero pad row
    h1e = np.zeros((N + 1, H + 1), BF16)
    s1p = np.zeros(N + 1, np.float32)
    d1p = np.zeros(N + 1, np.float32)
    for c in range(NCORES):
        sl = slice(c * CH, (c + 1) * CH)
        h1e[sl, :H] = r1[c]["hb"].T
        h1e[sl, H] = BF16(1.0)
        s1p[sl.start:sl.stop] = r1[c]["s1o"][0]
        d1p[sl.start:sl.stop] = r1[c]["d1o"][0]

    # ---- NEFF 2
    nc2 = _build_edge_neff(1, NW, G, Gtot, WT, H, K)
    b1f = np.asarray(b1, np.float32)
    c2v = (np.asarray(W2, np.float32).T @ b1f)[:, None]
    maps2 = []
    for c in range(NCORES):
        m = meta['cores'][c]
        gt1 = _expand(h1e, m['sidx']).reshape(128, Gtot * (H + 1))
        maps2.append({
            "gt": gt1, "O": m['O'],
            "se": _expand1(s1p, m['sidx']),
            "de": _expand1(d1p, m['didx']),
            "brep": np.tile(-b1f, (128, 1)),
            "W2": np.asarray(W2, np.float32),
            "a2s": np.asarray(a_src2, np.float32)[:, None],
            "a2d": np.asarray(a_dst2, np.float32)[:, None],
            "c2": c2v})
    print("[kernel] NEFF2 built, running...", file=sys.stderr, flush=True)
    _res2 = run_bass_kernel_spmd(nc2, maps2, cores)
    RESULTS.append(_res2)
    r2 = _res2.results
    print("[kernel] NEFF2 done", file=sys.stderr, flush=True)

    # host: full [h2|1] (bf16), s2, d2 tables
    h2p = np.zeros((N + 1, K + 1), FP8)
    s2p = np.zeros(N + 1, np.float32)
    d2p = np.zeros(N + 1, np.float32)
    for c in range(NCORES):
        sl = slice(c * CH, (c + 1) * CH)
        h2p[sl, :K] = r2[c]["h2o"][:, :CH].T.astype(FP8)
        h2p[sl, K] = FP8(1.0)
        s2p[sl.start:sl.stop] = r2[c]["s2o"][0, :CH]
        d2p[sl.start:sl.stop] = r2[c]["d2o"][0, :CH]

    # ---- NEFF 3
    nc3 = _build_edge_neff(2, NW, G, Gtot, WT, H, K)
    maps3 = []
    for c in range(NCORES):
        m = meta['cores'][c]
        gt2 = _expand(h2p, m['sidx']).reshape(128, Gtot * (K + 1))
        maps3.append({
            "gt": gt2, "O": m['O'],
            "se": _expand1(s2p, m['sidx']),
            "de": _expand1(d2p, m['didx']),
            "brep": np.tile(np.asarray(b2, np.float32), (128, 1))})
    print("[kernel] NEFF3 built, running...", file=sys.stderr, flush=True)
    _res3 = run_bass_kernel_spmd(nc3, maps3, cores)
    RESULTS.append(_res3)
    r3 = _res3.results
    print("[kernel] NEFF3 done", file=sys.stderr, flush=True)

    # unshuffle: r3[c]["out"][p, w*K:(w+1)*K] is node c*CH + w*WIN + p (p<WIN)
    out = np.empty((N, K), np.float32)
    for c in range(NCORES):
        sg = r3[c]["out"].reshape(128, NW, K).transpose(1, 0, 2)  # [w, p, K]
        out[c * CH:(c + 1) * CH] = sg[:, :WIN, :].reshape(NW * WIN, K)[:CH]
    return out
